# revision 12
# baseline (speedup 1.0000x reference)
"""2-layer GCN encoder on 8 Trainium2 NeuronCores (Bass/Tile).

Algorithm (per layer, using GCNConv linearity: A_hat @ (x @ W) == (A_hat @ x) @ W):
  dinv = 1/sqrt(deg+1);  htab = bf16(dinv * x_layer_input)     (node-sharded)
  AllGather htab -> full gather table in DRAM
  per dst-node tile of 128: dma_gather the htab rows of all in-edges (plus a
  self-loop edge per node), reduce into PSUM via one-hot matmuls
  (aggT[feat,dst] += msg_chunk^T @ onehot(dst_local)), then
  z = aggT^T @ W (natural layout via lhsT=aggT), post-scale by dinv, bias, relu.

Host side does only index preprocessing (edge partitioning/sorting/padding)
and sharding; all float math runs on device.
"""

import math
import numpy as np
import ml_dtypes
from contextlib import ExitStack

# ---- static problem config (hardcoded per contract) ----
N = 100000
E = 1600000
DIN = 128
DH = 128
DOUT = 64
NCORES = 8
NPC = N // NCORES            # 12500 nodes per core
NT = math.ceil(NPC / 128)    # 98 dst tiles per core
LAST_ROWS = NPC - (NT - 1) * 128   # 84
WIN = 32768                  # int16 index window for dma_gather
NW = math.ceil(N / WIN)      # 4 windows
SENTINEL = 200.0             # dst_local value for pad slots (matches no iota lane)

_CACHE = {}

# gather tuning knobs (A/B): GATHER_ELEM = elements fetched per descriptor
# (128 = exact row, 256 = 512B over-fetch covering rows i,i+1 for line-rate
# SDMA transfers); SINGLE_PACKET passed through to dma_gather.
GATHER_ELEM = 128
SINGLE_PACKET = False


def _preprocess(edge_index):
    """Partition/sort/pad edges. Returns per-core index arrays + shared schedule."""
    src = np.ascontiguousarray(edge_index[0]).astype(np.int64)
    dst = np.ascontiguousarray(edge_index[1]).astype(np.int64)

    deg = np.bincount(dst, minlength=N).astype(np.float64) + 1.0
    dinv = (1.0 / np.sqrt(deg)).astype(np.float32)

    per_core = []
    counts = np.zeros((NCORES, NT * NW), dtype=np.int64)
    for c in range(NCORES):
        lo, hi = c * NPC, (c + 1) * NPC
        sel = (dst >= lo) & (dst < hi)
        es = np.concatenate([src[sel], np.arange(lo, hi, dtype=np.int64)])
        ed = np.concatenate([dst[sel] - lo, np.arange(NPC, dtype=np.int64)])
        t = ed >> 7
        w = es // WIN
        gid = t * NW + w
        order = np.argsort(gid, kind="stable")
        es, ed, gid = es[order], ed[order], gid[order]
        counts[c] = np.bincount(gid, minlength=NT * NW)
        per_core.append((es, ed, gid))

    cnt_max = counts.max(axis=0)
    cnt_pad = ((cnt_max + 127) // 128) * 128          # 0 stays 0
    slot_off = np.zeros(NT * NW, dtype=np.int64)
    slot_off[1:] = np.cumsum(cnt_pad)[:-1]
    TOT = int(cnt_pad.sum())

    # shared gather schedule: per tile, list of (window, n_chunks, slot_off)
    sched = []
    for t in range(NT):
        ws = []
        for w in range(NW):
            g = t * NW + w
            if cnt_pad[g] > 0:
                ws.append((w, int(cnt_pad[g] // 128), int(slot_off[g])))
        sched.append(ws)

    idx_maps, dl_maps = [], []
    for c in range(NCORES):
        es, ed, gid = per_core[c]
        cstart = np.zeros(NT * NW, dtype=np.int64)
        cstart[1:] = np.cumsum(counts[c])[:-1]
        rank = np.arange(len(es)) - cstart[gid]
        slot = slot_off[gid] + rank
        idx = np.zeros(TOT, dtype=np.int16)
        dl = np.full(TOT, SENTINEL, dtype=np.float32)
        idx[slot] = (es - (es // WIN) * WIN).astype(np.int16)
        dl[slot] = (ed & 127).astype(np.float32)
        # SBUF layouts: idx wrapped over 16 partitions (replicated x8),
        # dstloc wrapped over 128 partitions, one column per 128-edge chunk.
        idx_sb = np.tile(np.ascontiguousarray(idx.reshape(-1, 16).T), (8, 1))
        dl_sb = np.ascontiguousarray(dl.reshape(-1, 128).T).astype(ml_dtypes.bfloat16)
        idx_maps.append(idx_sb)
        dl_maps.append(dl_sb)

    return dinv, idx_maps, dl_maps, sched, TOT


def _build(sched, TOT, b1_nz, b2_nz):
    import concourse.bass as bass
    import concourse.tile as tile
    from concourse import bacc, mybir

    f32 = mybir.dt.float32
    bf16 = mybir.dt.bfloat16
    AF = mybir.ActivationFunctionType
    OP = mybir.AluOpType

    nc = bacc.Bacc("TRN2", target_bir_lowering=False, debug=False,
                   num_devices=NCORES, num_swdge_queues=4)
    qctr = [0]

    x_d = nc.dram_tensor("x", [NT * 128, DIN], f32, kind="ExternalInput").ap()
    dinv_d = nc.dram_tensor("dinv", [128, NT], f32, kind="ExternalInput").ap()
    idx_d = nc.dram_tensor("idx", [128, TOT // 16], mybir.dt.int16,
                           kind="ExternalInput").ap()
    dl_d = nc.dram_tensor("dstloc", [128, TOT // 128], bf16,
                          kind="ExternalInput").ap()
    W1_d = nc.dram_tensor("W1", [DIN, DH], f32, kind="ExternalInput").ap()
    W2_d = nc.dram_tensor("W2", [DH, DOUT], f32, kind="ExternalInput").ap()
    b1_d = nc.dram_tensor("b1", [128, DH], f32, kind="ExternalInput").ap()
    b2_d = nc.dram_tensor("b2", [128, DOUT], f32, kind="ExternalInput").ap()
    out_d = nc.dram_tensor("out", [NPC, DOUT], f32, kind="ExternalOutput").ap()

    groups = [list(range(NCORES))]

    with tile.TileContext(nc) as tc, ExitStack() as ctx:
        dram = ctx.enter_context(tc.tile_pool(name="dram", bufs=1, space="DRAM"))
        # full tables padded by 128 rows so 512B over-fetch descriptors
        # (rows i,i+1) stay in-bounds at the last node
        tab1_shard = dram.tile([NPC, DIN], bf16)
        tab1_full = dram.tile([N + 128, DIN], bf16, addr_space="Shared")
        tab2_shard = dram.tile([NPC, DH], bf16)
        tab2_full = dram.tile([N + 128, DH], bf16, addr_space="Shared")

        const = ctx.enter_context(tc.tile_pool(name="const", bufs=1))
        xpool = ctx.enter_context(tc.tile_pool(name="xp", bufs=3))
        hpool = ctx.enter_context(tc.tile_pool(name="hp", bufs=3))
        msgpool = ctx.enter_context(tc.tile_pool(name="msg", bufs=3))
        mpool = ctx.enter_context(tc.tile_pool(name="mm", bufs=3))
        cppool = ctx.enter_context(tc.tile_pool(name="cp", bufs=3))
        upool = ctx.enter_context(tc.tile_pool(name="up", bufs=3))
        psA = ctx.enter_context(tc.tile_pool(name="psA", bufs=2, space="PSUM"))
        psB = ctx.enter_context(tc.tile_pool(name="psB", bufs=2, space="PSUM"))

        # ---- constants ----
        iota_i = const.tile([128, 128], mybir.dt.int32)
        nc.gpsimd.iota(iota_i[:], pattern=[[1, 128]], base=0, channel_multiplier=0)
        iota_b = const.tile([128, 128], bf16)
        nc.vector.tensor_copy(iota_b[:], iota_i[:])

        dinv_t = const.tile([128, NT], f32)
        nc.sync.dma_start(dinv_t[:], dinv_d[:])
        idx_t = const.tile([128, TOT // 16], mybir.dt.int16)
        nc.sync.dma_start(idx_t[:], idx_d[:])
        dl_t = const.tile([128, TOT // 128], bf16)
        nc.sync.dma_start(dl_t[:], dl_d[:])

        W1f = const.tile([DIN, DH], f32)
        nc.sync.dma_start(W1f[:], W1_d[:])
        W1b = const.tile([DIN, DH], bf16)
        nc.vector.tensor_copy(W1b[:], W1f[:])
        W2f = const.tile([DH, DOUT], f32)
        nc.sync.dma_start(W2f[:], W2_d[:])
        W2b = const.tile([DH, DOUT], bf16)
        nc.vector.tensor_copy(W2b[:], W2f[:])
        if b1_nz:
            b1r = const.tile([128, DH], f32)
            nc.sync.dma_start(b1r[:], b1_d[:])
        if b2_nz:
            b2r = const.tile([128, DOUT], f32)
            nc.sync.dma_start(b2r[:], b2_d[:])

        def rows_of(t):
            return LAST_ROWS if t == NT - 1 else 128

        # ---- phase 1: layer-1 gather table (h1 = dinv * x, bf16) ----
        for t in range(NT):
            xt = xpool.tile([128, DIN], f32, tag="xt")
            nc.sync.dma_start(xt[:], x_d[t * 128:(t + 1) * 128, :])
            h1 = hpool.tile([128, DIN], bf16, tag="h1")
            nc.scalar.activation(h1[:], xt[:], AF.Copy, scale=dinv_t[:, t:t + 1])
            r = rows_of(t)
            nc.sync.dma_start(tab1_shard[t * 128:t * 128 + r, :], h1[0:r, :])

        nc.gpsimd.collective_compute(
            "AllGather", OP.bypass, replica_groups=groups,
            ins=[tab1_shard[:].opt()], outs=[tab1_full[0:N, :].opt()])

        def aggregate(t, tab_full, dim):
            """Gather in-edge rows for dst tile t and reduce into PSUM.
            Returns aggT PSUM tile [dim(feat), 128(dst)]."""
            ws = sched[t]
            CH = sum(nch for (_, nch, _) in ws)
            ge = GATHER_ELEM if dim == 128 else dim
            msg = msgpool.tile([128, CH, ge], bf16, tag="msg")
            cum = 0
            for (w, nch, soff) in ws:
                wrows = min(WIN, N - w * WIN)
                t_ap = tab_full[w * WIN:w * WIN + wrows, :]
                if ge != dim:
                    in_ap = bass.AP(t_ap.tensor, t_ap.offset,
                                    [[dim, wrows], [1, ge]])
                else:
                    in_ap = t_ap
                nc.gpsimd.dma_gather(
                    msg[:, cum:cum + nch, :],
                    in_ap,
                    idx_t[:, soff // 16: soff // 16 + nch * 8],
                    num_idxs=nch * 128,
                    num_idxs_reg=nch * 128,
                    elem_size=ge,
                    elem_step=dim if ge != dim else None,
                    single_packet=SINGLE_PACKET,
                    queue_num=qctr[0] % 4,
                )
                qctr[0] += 1
                cum += nch
            # one-hot M: [128(edge), CH*128(dst-lane)]
            cb = None
            for (w, nch, soff) in ws:
                if cb is None:
                    cb = soff // 128
            M = mpool.tile([128, CH * 128], bf16, tag="M")
            m_ap = M[:]
            out3 = bass.AP(m_ap.tensor, m_ap.offset,
                           [list(m_ap.ap[0]), [128, CH], [1, 128]])
            in0 = dl_t[:, cb:cb + CH].to_broadcast([128, CH, 128])
            io_ap = iota_b[:]
            in1 = bass.AP(io_ap.tensor, io_ap.offset,
                          [list(io_ap.ap[0]), [0, CH], [1, 128]])
            nc.vector.tensor_tensor(out3, in0, in1, op=OP.is_equal)

            agg = psA.tile([dim, 128], f32, tag="agg")
            for k in range(CH):
                nc.tensor.matmul(
                    out=agg[:],
                    lhsT=msg[:, k:k + 1, 0:dim].opt(),
                    rhs=M[:, k * 128:(k + 1) * 128],
                    start=(k == 0), stop=(k == CH - 1))
            return agg

        # ---- phase 2: layer 1 aggregate + transform -> layer-2 table ----
        for t in range(NT):
            agg = aggregate(t, tab1_full, DIN)
            cp = cppool.tile([DIN, 128], bf16, tag="cp")
            nc.scalar.activation(cp[:], agg[:], AF.Copy)
            z1 = psB.tile([128, DH], f32, tag="z1")
            nc.tensor.matmul(out=z1[:], lhsT=cp[:], rhs=W1b[:],
                             start=True, stop=True)
            # h2 = dinv * relu(dinv * z1 + b1)  (dinv per-partition here)
            if b1_nz:
                u = upool.tile([128, DH], f32, tag="u")
                nc.scalar.activation(u[:], z1[:], AF.Copy,
                                     scale=dinv_t[:, t:t + 1])
                v = upool.tile([128, DH], f32, tag="v")
                nc.vector.tensor_tensor(v[:], u[:], b1r[:], op=OP.add)
                h2 = hpool.tile([128, DH], bf16, tag="h2")
                nc.scalar.activation(h2[:], v[:], AF.Relu,
                                     scale=dinv_t[:, t:t + 1])
            else:
                u = upool.tile([128, DH], f32, tag="u")
                nc.scalar.activation(u[:], z1[:], AF.Copy,
                                     scale=dinv_t[:, t:t + 1])
                h2 = hpool.tile([128, DH], bf16, tag="h2")
                nc.scalar.activation(h2[:], u[:], AF.Relu,
                                     scale=dinv_t[:, t:t + 1])
            r = rows_of(t)
            nc.sync.dma_start(tab2_shard[t * 128:t * 128 + r, :], h2[0:r, :])

        nc.gpsimd.collective_compute(
            "AllGather", OP.bypass, replica_groups=groups,
            ins=[tab2_shard[:].opt()], outs=[tab2_full[0:N, :].opt()])

        # ---- phase 3: layer 2 aggregate + transform -> output ----
        for t in range(NT):
            agg = aggregate(t, tab2_full, DH)
            cp = cppool.tile([DH, 128], bf16, tag="cp")
            nc.scalar.activation(cp[:], agg[:], AF.Copy)
            z2 = psB.tile([128, DOUT], f32, tag="z2")
            nc.tensor.matmul(out=z2[:], lhsT=cp[:], rhs=W2b[:],
                             start=True, stop=True)
            u2 = upool.tile([128, DOUT], f32, tag="u2")
            nc.scalar.activation(u2[:], z2[:], AF.Copy,
                                 scale=dinv_t[:, t:t + 1])
            if b2_nz:
                v2 = upool.tile([128, DOUT], f32, tag="v2")
                nc.vector.tensor_tensor(v2[:], u2[:], b2r[:], op=OP.add)
                fin = v2
            else:
                fin = u2
            r = rows_of(t)
            nc.sync.dma_start(out_d[t * 128:t * 128 + r, :], fin[0:r, :])

    nc.compile()
    return nc


def kernel(x, edge_index, W1, b1, W2, b2):
    from concourse.bass_utils import run_bass_kernel_spmd

    x = np.asarray(x, dtype=np.float32)
    W1 = np.asarray(W1, dtype=np.float32)
    W2 = np.asarray(W2, dtype=np.float32)
    b1 = np.asarray(b1, dtype=np.float32)
    b2 = np.asarray(b2, dtype=np.float32)
    ei = np.asarray(edge_index)

    dinv, idx_maps, dl_maps, sched, TOT = _preprocess(ei)

    b1_nz = bool(np.any(b1 != 0))
    b2_nz = bool(np.any(b2 != 0))
    key = ("graph", TOT, tuple(tuple(w) for ws in sched for w in ws),
           b1_nz, b2_nz, GATHER_ELEM, SINGLE_PACKET)
    if key not in _CACHE:
        _CACHE.clear()
        _CACHE[key] = _build(sched, TOT, b1_nz, b2_nz)
    nc = _CACHE[key]

    b1r = np.broadcast_to(b1.reshape(1, DH), (128, DH)).copy()
    b2r = np.broadcast_to(b2.reshape(1, DOUT), (128, DOUT)).copy()

    in_maps = []
    for c in range(NCORES):
        lo, hi = c * NPC, (c + 1) * NPC
        xs = np.zeros((NT * 128, DIN), dtype=np.float32)
        xs[:NPC] = x[lo:hi]
        dv = np.zeros((128, NT), dtype=np.float32)
        dvflat = np.zeros(NT * 128, dtype=np.float32)
        dvflat[:NPC] = dinv[lo:hi]
        dv[:] = dvflat.reshape(NT, 128).T
        in_maps.append({
            "x": xs, "dinv": dv,
            "idx": idx_maps[c], "dstloc": dl_maps[c],
            "W1": W1, "W2": W2, "b1": b1r, "b2": b2r,
        })

    res = run_bass_kernel_spmd(nc, in_maps, list(range(NCORES)))
    globals()["LAST_RESULTS"] = res
    out = np.concatenate([res.results[c]["out"] for c in range(NCORES)], axis=0)
    return out.astype(np.float32)



# revision 13
# speedup vs baseline: 1.0778x; 1.0778x over previous
"""2-layer GCN encoder on 8 Trainium2 NeuronCores (Bass/Tile).

Algorithm (per layer, using GCNConv linearity: A_hat @ (x @ W) == (A_hat @ x) @ W):
  dinv = 1/sqrt(deg+1);  htab = bf16(dinv * x_layer_input)     (node-sharded)
  AllGather htab -> full gather table in DRAM
  per dst-node tile of 128: dma_gather the htab rows of all in-edges (plus a
  self-loop edge per node), reduce into PSUM via one-hot matmuls
  (aggT[feat,dst] += msg_chunk^T @ onehot(dst_local)), then
  z = aggT^T @ W (natural layout via lhsT=aggT), post-scale by dinv, bias, relu.

Host side does only index preprocessing (edge partitioning/sorting/padding)
and sharding; all float math runs on device.
"""

import math
import numpy as np
import ml_dtypes
from contextlib import ExitStack

# ---- static problem config (hardcoded per contract) ----
N = 100000
E = 1600000
DIN = 128
DH = 128
DOUT = 64
NCORES = 8
NPC = N // NCORES            # 12500 nodes per core
NT = math.ceil(NPC / 128)    # 98 dst tiles per core
LAST_ROWS = NPC - (NT - 1) * 128   # 84
WIN = 32768                  # int16 index window for dma_gather
NW = math.ceil(N / WIN)      # 4 windows
SENTINEL = 200.0             # dst_local value for pad slots (matches no iota lane)

_CACHE = {}

# gather tuning knobs (A/B): GATHER_ELEM = elements fetched per descriptor
# (128 = exact row, 256 = 512B over-fetch covering rows i,i+1 for line-rate
# SDMA transfers); SINGLE_PACKET passed through to dma_gather.
GATHER_ELEM = 128
SINGLE_PACKET = True


def _preprocess(edge_index):
    """Partition/sort/pad edges. Returns per-core index arrays + shared schedule."""
    src = np.ascontiguousarray(edge_index[0]).astype(np.int64)
    dst = np.ascontiguousarray(edge_index[1]).astype(np.int64)

    deg = np.bincount(dst, minlength=N).astype(np.float64) + 1.0
    dinv = (1.0 / np.sqrt(deg)).astype(np.float32)

    per_core = []
    counts = np.zeros((NCORES, NT * NW), dtype=np.int64)
    for c in range(NCORES):
        lo, hi = c * NPC, (c + 1) * NPC
        sel = (dst >= lo) & (dst < hi)
        es = np.concatenate([src[sel], np.arange(lo, hi, dtype=np.int64)])
        ed = np.concatenate([dst[sel] - lo, np.arange(NPC, dtype=np.int64)])
        t = ed >> 7
        w = es // WIN
        gid = t * NW + w
        order = np.argsort(gid, kind="stable")
        es, ed, gid = es[order], ed[order], gid[order]
        counts[c] = np.bincount(gid, minlength=NT * NW)
        per_core.append((es, ed, gid))

    cnt_max = counts.max(axis=0)
    cnt_pad = ((cnt_max + 127) // 128) * 128          # 0 stays 0
    slot_off = np.zeros(NT * NW, dtype=np.int64)
    slot_off[1:] = np.cumsum(cnt_pad)[:-1]
    TOT = int(cnt_pad.sum())

    # shared gather schedule: per tile, list of (window, n_chunks, slot_off)
    sched = []
    for t in range(NT):
        ws = []
        for w in range(NW):
            g = t * NW + w
            if cnt_pad[g] > 0:
                ws.append((w, int(cnt_pad[g] // 128), int(slot_off[g])))
        sched.append(ws)

    idx_maps, dl_maps = [], []
    for c in range(NCORES):
        es, ed, gid = per_core[c]
        cstart = np.zeros(NT * NW, dtype=np.int64)
        cstart[1:] = np.cumsum(counts[c])[:-1]
        rank = np.arange(len(es)) - cstart[gid]
        slot = slot_off[gid] + rank
        idx = np.zeros(TOT, dtype=np.int16)
        dl = np.full(TOT, SENTINEL, dtype=np.float32)
        idx[slot] = (es - (es // WIN) * WIN).astype(np.int16)
        dl[slot] = (ed & 127).astype(np.float32)
        # SBUF layouts: idx wrapped over 16 partitions (replicated x8),
        # dstloc wrapped over 128 partitions, one column per 128-edge chunk.
        idx_sb = np.tile(np.ascontiguousarray(idx.reshape(-1, 16).T), (8, 1))
        dl_sb = np.ascontiguousarray(dl.reshape(-1, 128).T).astype(ml_dtypes.bfloat16)
        idx_maps.append(idx_sb)
        dl_maps.append(dl_sb)

    return dinv, idx_maps, dl_maps, sched, TOT


def _build(sched, TOT, b1_nz, b2_nz):
    import concourse.bass as bass
    import concourse.tile as tile
    from concourse import bacc, mybir

    f32 = mybir.dt.float32
    bf16 = mybir.dt.bfloat16
    AF = mybir.ActivationFunctionType
    OP = mybir.AluOpType

    nc = bacc.Bacc("TRN2", target_bir_lowering=False, debug=False,
                   num_devices=NCORES, num_swdge_queues=2)
    qctr = [0]

    x_d = nc.dram_tensor("x", [NT * 128, DIN], f32, kind="ExternalInput").ap()
    dinv_d = nc.dram_tensor("dinv", [128, NT], f32, kind="ExternalInput").ap()
    idx_d = nc.dram_tensor("idx", [128, TOT // 16], mybir.dt.int16,
                           kind="ExternalInput").ap()
    dl_d = nc.dram_tensor("dstloc", [128, TOT // 128], bf16,
                          kind="ExternalInput").ap()
    W1_d = nc.dram_tensor("W1", [DIN, DH], f32, kind="ExternalInput").ap()
    W2_d = nc.dram_tensor("W2", [DH, DOUT], f32, kind="ExternalInput").ap()
    b1_d = nc.dram_tensor("b1", [128, DH], f32, kind="ExternalInput").ap()
    b2_d = nc.dram_tensor("b2", [128, DOUT], f32, kind="ExternalInput").ap()
    out_d = nc.dram_tensor("out", [NPC, DOUT], f32, kind="ExternalOutput").ap()

    groups = [list(range(NCORES))]

    with tile.TileContext(nc) as tc, ExitStack() as ctx:
        dram = ctx.enter_context(tc.tile_pool(name="dram", bufs=1, space="DRAM"))
        # full tables padded by 128 rows so 512B over-fetch descriptors
        # (rows i,i+1) stay in-bounds at the last node
        tab1_shard = dram.tile([NPC, DIN], bf16)
        tab1_full = dram.tile([N + 128, DIN], bf16, addr_space="Shared")
        tab2_shard = dram.tile([NPC, DH], bf16)
        tab2_full = dram.tile([N + 128, DH], bf16, addr_space="Shared")

        const = ctx.enter_context(tc.tile_pool(name="const", bufs=1))
        xpool = ctx.enter_context(tc.tile_pool(name="xp", bufs=3))
        hpool = ctx.enter_context(tc.tile_pool(name="hp", bufs=3))
        msgpool = ctx.enter_context(tc.tile_pool(name="msg", bufs=3))
        mpool = ctx.enter_context(tc.tile_pool(name="mm", bufs=3))
        cppool = ctx.enter_context(tc.tile_pool(name="cp", bufs=3))
        upool = ctx.enter_context(tc.tile_pool(name="up", bufs=3))
        psA = ctx.enter_context(tc.tile_pool(name="psA", bufs=2, space="PSUM"))
        psB = ctx.enter_context(tc.tile_pool(name="psB", bufs=2, space="PSUM"))

        # ---- constants ----
        iota_i = const.tile([128, 128], mybir.dt.int32)
        nc.gpsimd.iota(iota_i[:], pattern=[[1, 128]], base=0, channel_multiplier=0)
        iota_b = const.tile([128, 128], bf16)
        nc.vector.tensor_copy(iota_b[:], iota_i[:])

        dinv_t = const.tile([128, NT], f32)
        nc.sync.dma_start(dinv_t[:], dinv_d[:])
        idx_t = const.tile([128, TOT // 16], mybir.dt.int16)
        nc.sync.dma_start(idx_t[:], idx_d[:])
        dl_t = const.tile([128, TOT // 128], bf16)
        nc.sync.dma_start(dl_t[:], dl_d[:])

        W1f = const.tile([DIN, DH], f32)
        nc.sync.dma_start(W1f[:], W1_d[:])
        W1b = const.tile([DIN, DH], bf16)
        nc.vector.tensor_copy(W1b[:], W1f[:])
        W2f = const.tile([DH, DOUT], f32)
        nc.sync.dma_start(W2f[:], W2_d[:])
        W2b = const.tile([DH, DOUT], bf16)
        nc.vector.tensor_copy(W2b[:], W2f[:])
        if b1_nz:
            b1r = const.tile([128, DH], f32)
            nc.sync.dma_start(b1r[:], b1_d[:])
        if b2_nz:
            b2r = const.tile([128, DOUT], f32)
            nc.sync.dma_start(b2r[:], b2_d[:])

        def rows_of(t):
            return LAST_ROWS if t == NT - 1 else 128

        # ---- phase 1: layer-1 gather table (h1 = dinv * x, bf16) ----
        for t in range(NT):
            xt = xpool.tile([128, DIN], f32, tag="xt")
            nc.sync.dma_start(xt[:], x_d[t * 128:(t + 1) * 128, :])
            h1 = hpool.tile([128, DIN], bf16, tag="h1")
            nc.scalar.activation(h1[:], xt[:], AF.Copy, scale=dinv_t[:, t:t + 1])
            r = rows_of(t)
            nc.sync.dma_start(tab1_shard[t * 128:t * 128 + r, :], h1[0:r, :])

        nc.gpsimd.collective_compute(
            "AllGather", OP.bypass, replica_groups=groups,
            ins=[tab1_shard[:].opt()], outs=[tab1_full[0:N, :].opt()])

        def aggregate(t, tab_full, dim):
            """Gather in-edge rows for dst tile t and reduce into PSUM.
            Returns aggT PSUM tile [dim(feat), 128(dst)]."""
            ws = sched[t]
            CH = sum(nch for (_, nch, _) in ws)
            ge = GATHER_ELEM if dim == 128 else dim
            msg = msgpool.tile([128, CH, ge], bf16, tag="msg")
            cum = 0
            for (w, nch, soff) in ws:
                wrows = min(WIN, N - w * WIN)
                t_ap = tab_full[w * WIN:w * WIN + wrows, :]
                if ge != dim:
                    in_ap = bass.AP(t_ap.tensor, t_ap.offset,
                                    [[dim, wrows], [1, ge]])
                else:
                    in_ap = t_ap
                nc.gpsimd.dma_gather(
                    msg[:, cum:cum + nch, :],
                    in_ap,
                    idx_t[:, soff // 16: soff // 16 + nch * 8],
                    num_idxs=nch * 128,
                    num_idxs_reg=nch * 128,
                    elem_size=ge,
                    elem_step=dim if ge != dim else None,
                    single_packet=SINGLE_PACKET,
                    queue_num=qctr[0] % 2,
                )
                qctr[0] += 1
                cum += nch
            # one-hot M: [128(edge), CH*128(dst-lane)]
            cb = None
            for (w, nch, soff) in ws:
                if cb is None:
                    cb = soff // 128
            M = mpool.tile([128, CH * 128], bf16, tag="M")
            m_ap = M[:]
            out3 = bass.AP(m_ap.tensor, m_ap.offset,
                           [list(m_ap.ap[0]), [128, CH], [1, 128]])
            in0 = dl_t[:, cb:cb + CH].to_broadcast([128, CH, 128])
            io_ap = iota_b[:]
            in1 = bass.AP(io_ap.tensor, io_ap.offset,
                          [list(io_ap.ap[0]), [0, CH], [1, 128]])
            nc.vector.tensor_tensor(out3, in0, in1, op=OP.is_equal)

            agg = psA.tile([dim, 128], f32, tag="agg")
            for k in range(CH):
                nc.tensor.matmul(
                    out=agg[:],
                    lhsT=msg[:, k:k + 1, 0:dim].opt(),
                    rhs=M[:, k * 128:(k + 1) * 128],
                    start=(k == 0), stop=(k == CH - 1))
            return agg

        # ---- phase 2: layer 1 aggregate + transform -> layer-2 table ----
        for t in range(NT):
            agg = aggregate(t, tab1_full, DIN)
            cp = cppool.tile([DIN, 128], bf16, tag="cp")
            nc.scalar.activation(cp[:], agg[:], AF.Copy)
            z1 = psB.tile([128, DH], f32, tag="z1")
            nc.tensor.matmul(out=z1[:], lhsT=cp[:], rhs=W1b[:],
                             start=True, stop=True)
            # h2 = dinv * relu(dinv * z1 + b1)  (dinv per-partition here)
            if b1_nz:
                u = upool.tile([128, DH], f32, tag="u")
                nc.scalar.activation(u[:], z1[:], AF.Copy,
                                     scale=dinv_t[:, t:t + 1])
                v = upool.tile([128, DH], f32, tag="v")
                nc.vector.tensor_tensor(v[:], u[:], b1r[:], op=OP.add)
                h2 = hpool.tile([128, DH], bf16, tag="h2")
                nc.scalar.activation(h2[:], v[:], AF.Relu,
                                     scale=dinv_t[:, t:t + 1])
            else:
                u = upool.tile([128, DH], f32, tag="u")
                nc.scalar.activation(u[:], z1[:], AF.Copy,
                                     scale=dinv_t[:, t:t + 1])
                h2 = hpool.tile([128, DH], bf16, tag="h2")
                nc.scalar.activation(h2[:], u[:], AF.Relu,
                                     scale=dinv_t[:, t:t + 1])
            r = rows_of(t)
            nc.sync.dma_start(tab2_shard[t * 128:t * 128 + r, :], h2[0:r, :])

        nc.gpsimd.collective_compute(
            "AllGather", OP.bypass, replica_groups=groups,
            ins=[tab2_shard[:].opt()], outs=[tab2_full[0:N, :].opt()])

        # ---- phase 3: layer 2 aggregate + transform -> output ----
        for t in range(NT):
            agg = aggregate(t, tab2_full, DH)
            cp = cppool.tile([DH, 128], bf16, tag="cp")
            nc.scalar.activation(cp[:], agg[:], AF.Copy)
            z2 = psB.tile([128, DOUT], f32, tag="z2")
            nc.tensor.matmul(out=z2[:], lhsT=cp[:], rhs=W2b[:],
                             start=True, stop=True)
            u2 = upool.tile([128, DOUT], f32, tag="u2")
            nc.scalar.activation(u2[:], z2[:], AF.Copy,
                                 scale=dinv_t[:, t:t + 1])
            if b2_nz:
                v2 = upool.tile([128, DOUT], f32, tag="v2")
                nc.vector.tensor_tensor(v2[:], u2[:], b2r[:], op=OP.add)
                fin = v2
            else:
                fin = u2
            r = rows_of(t)
            nc.sync.dma_start(out_d[t * 128:t * 128 + r, :], fin[0:r, :])

    nc.compile()
    return nc


def kernel(x, edge_index, W1, b1, W2, b2):
    from concourse.bass_utils import run_bass_kernel_spmd

    x = np.asarray(x, dtype=np.float32)
    W1 = np.asarray(W1, dtype=np.float32)
    W2 = np.asarray(W2, dtype=np.float32)
    b1 = np.asarray(b1, dtype=np.float32)
    b2 = np.asarray(b2, dtype=np.float32)
    ei = np.asarray(edge_index)

    dinv, idx_maps, dl_maps, sched, TOT = _preprocess(ei)

    b1_nz = bool(np.any(b1 != 0))
    b2_nz = bool(np.any(b2 != 0))
    key = ("graph", TOT, tuple(tuple(w) for ws in sched for w in ws),
           b1_nz, b2_nz, GATHER_ELEM, SINGLE_PACKET)
    if key not in _CACHE:
        _CACHE.clear()
        _CACHE[key] = _build(sched, TOT, b1_nz, b2_nz)
    nc = _CACHE[key]

    b1r = np.broadcast_to(b1.reshape(1, DH), (128, DH)).copy()
    b2r = np.broadcast_to(b2.reshape(1, DOUT), (128, DOUT)).copy()

    in_maps = []
    for c in range(NCORES):
        lo, hi = c * NPC, (c + 1) * NPC
        xs = np.zeros((NT * 128, DIN), dtype=np.float32)
        xs[:NPC] = x[lo:hi]
        dv = np.zeros((128, NT), dtype=np.float32)
        dvflat = np.zeros(NT * 128, dtype=np.float32)
        dvflat[:NPC] = dinv[lo:hi]
        dv[:] = dvflat.reshape(NT, 128).T
        in_maps.append({
            "x": xs, "dinv": dv,
            "idx": idx_maps[c], "dstloc": dl_maps[c],
            "W1": W1, "W2": W2, "b1": b1r, "b2": b2r,
        })

    res = run_bass_kernel_spmd(nc, in_maps, list(range(NCORES)))
    globals()["LAST_RESULTS"] = res
    out = np.concatenate([res.results[c]["out"] for c in range(NCORES)], axis=0)
    return out.astype(np.float32)

